# revision 1
# baseline (speedup 1.0000x reference)
import sys

sys.path.insert(0, "/opt/trn_rl_repo")

import numpy as np
import concourse.bass as bass
import concourse.bacc as bacc
import concourse.mybir as mybir
from concourse.tile import TileContext
from concourse.bass_utils import run_bass_kernel_spmd

B, T, D = 2048, 200, 64
H1, H2 = 128, 64
NCORES = 8
BLOC = B // NCORES  # 256 batches per core
NBLK = BLOC // 128  # 2 blocks of 128 batches

F32 = mybir.dt.float32
AF = mybir.ActivationFunctionType
ALU = mybir.AluOpType
AX = mybir.AxisListType

_cached = None


def build_nc():
    nc = bacc.Bacc()
    qT_e = nc.declare_dram_parameter("qT", [D, BLOC], F32, isOutput=False)
    kT_e = nc.declare_dram_parameter("kT", [BLOC, D, T], F32, isOutput=False)
    v_e = nc.declare_dram_parameter("v", [BLOC, T, D], F32, isOutput=False)
    m_e = nc.declare_dram_parameter("maskf", [BLOC, T], F32, isOutput=False)
    Wk_e = nc.declare_dram_parameter("Wk", [D, H1], F32, isOutput=False)
    Wd_e = nc.declare_dram_parameter("Wd", [D, H1], F32, isOutput=False)
    Wq_e = nc.declare_dram_parameter("Wq", [D, H1], F32, isOutput=False)
    b1_e = nc.declare_dram_parameter("b1", [H1, 1], F32, isOutput=False)
    W2_e = nc.declare_dram_parameter("W2", [H1, H2], F32, isOutput=False)
    b2_e = nc.declare_dram_parameter("b2", [H2, 1], F32, isOutput=False)
    Wo_e = nc.declare_dram_parameter("Wo", [H2, 1], F32, isOutput=False)
    id_e = nc.declare_dram_parameter("ident", [128, 128], F32, isOutput=False)
    out_e = nc.declare_dram_parameter("out", [BLOC, D], F32, isOutput=True)

    with TileContext(nc) as tc:
        with (
            tc.tile_pool(name="const", bufs=1) as cp,
            tc.tile_pool(name="kpool", bufs=3) as kp,
            tc.tile_pool(name="wpool", bufs=3) as wp,
            tc.tile_pool(name="hpool", bufs=3) as hp,
            tc.tile_pool(name="vpool", bufs=4) as vp,
            tc.tile_pool(name="bpool", bufs=2) as bp,
            tc.tile_pool(name="spool", bufs=2) as sp,
            tc.tile_pool(name="ps_h1", bufs=2, space="PSUM") as ph1,
            tc.tile_pool(name="ps_h2", bufs=2, space="PSUM") as ph2,
            tc.tile_pool(name="ps_acc", bufs=1, space="PSUM") as pac,
            tc.tile_pool(name="ps_nat", bufs=1, space="PSUM") as pna,
            tc.tile_pool(name="ps_ot", bufs=1, space="PSUM") as pot,
        ):
            # ---- constants ----
            Wk_s = cp.tile([D, H1], F32, tag="Wk")
            nc.sync.dma_start(out=Wk_s[:, :], in_=Wk_e[:, :])
            Wd_s = cp.tile([D, H1], F32, tag="Wd")
            nc.sync.dma_start(out=Wd_s[:, :], in_=Wd_e[:, :])
            Wq_s = cp.tile([D, H1], F32, tag="Wq")
            nc.sync.dma_start(out=Wq_s[:, :], in_=Wq_e[:, :])
            W2_s = cp.tile([H1, H2], F32, tag="W2")
            nc.sync.dma_start(out=W2_s[:, :], in_=W2_e[:, :])
            Wo_s = cp.tile([H2, 1], F32, tag="Wo")
            nc.sync.dma_start(out=Wo_s[:, :], in_=Wo_e[:, :])
            b1_s = cp.tile([H1, 1], F32, tag="b1")
            nc.sync.dma_start(out=b1_s[:, :], in_=b1_e[:, :])
            b2_s = cp.tile([H2, 1], F32, tag="b2")
            nc.sync.dma_start(out=b2_s[:, :], in_=b2_e[:, :])
            qT_s = cp.tile([D, BLOC], F32, tag="qT")
            nc.sync.dma_start(out=qT_s[:, :], in_=qT_e[:, :])
            id_s = cp.tile([128, 128], F32, tag="ident")
            nc.sync.dma_start(out=id_s[:, :], in_=id_e[:, :])

            # C = Wq.T @ qT + b1   -> [H1, BLOC] per-batch bias for layer 1
            C_ps = ph1.tile([H1, BLOC], F32, tag="h1")
            nc.tensor.matmul(C_ps[:, :], Wq_s[:, :], qT_s[:, :], start=True, stop=True)
            C_s = cp.tile([H1, BLOC], F32, tag="C")
            nc.vector.tensor_scalar_add(C_s[:, :], C_ps[:, :], b1_s[:, 0:1])

            for blk in range(NBLK):
                scT_hi = pac.tile([128, 128], F32, tag="accA")  # [t0:128, j]
                scT_lo = pac.tile([128, 128], F32, tag="accB")  # [t128:200, j]
                for j in range(128):
                    b = blk * 128 + j
                    kT_b = kp.tile([D, T], F32, tag="kT")
                    nc.sync.dma_start(out=kT_b[:, :], in_=kT_e[b, :, :])
                    # effective W for k-path: Wk + diag(q_b) @ W1d
                    wke = wp.tile([D, H1], F32, tag="wke")
                    nc.vector.scalar_tensor_tensor(
                        wke[:, :], Wd_s[:, :], qT_s[:, b : b + 1], Wk_s[:, :],
                        op0=ALU.mult, op1=ALU.add,
                    )
                    h1p = ph1.tile([H1, T], F32, tag="h1")
                    nc.tensor.matmul(h1p[:, :], wke[:, :], kT_b[:, :], start=True, stop=True)
                    h1s = hp.tile([H1, T], F32, tag="h1s")
                    nc.vector.tensor_scalar(
                        h1s[:, :], h1p[:, :], C_s[:, b : b + 1], 0.0, op0=ALU.add, op1=ALU.max
                    )
                    h2p = ph2.tile([H2, T], F32, tag="h2")
                    nc.tensor.matmul(h2p[:, :], W2_s[:, :], h1s[:, :], start=True, stop=True)
                    h2s = hp.tile([H2, T], F32, tag="h2s")
                    nc.scalar.activation(
                        h2s[:, :], h2p[:, :], AF.Relu, bias=b2_s[:, 0:1], scale=1.0
                    )
                    # scores as columns of [t, j] PSUM tiles
                    nc.tensor.matmul(
                        scT_hi[:, j : j + 1], h2s[:, 0:128], Wo_s[:, :], start=True, stop=True
                    )
                    nc.tensor.matmul(
                        scT_lo[0:72, j : j + 1], h2s[:, 128:200], Wo_s[:, :], start=True, stop=True
                    )

                # ---- batched softmax over the block ----
                scTh_s = bp.tile([128, 128], F32, tag="scTh")
                nc.vector.tensor_copy(scTh_s[:, :], scT_hi[:, :])
                scTl_s = bp.tile([128, 128], F32, tag="scTl")
                nc.vector.tensor_copy(scTl_s[0:72, :], scT_lo[0:72, :])
                scp = pna.tile([128, T], F32, tag="nat")  # [b, t]
                nc.tensor.transpose(scp[:, 0:128], scTh_s[:, :], id_s[:, :])
                nc.tensor.transpose(scp[:, 128:200], scTl_s[0:72, :], id_s[0:72, 0:72])

                mask_s = sp.tile([128, T], F32, tag="mask")
                nc.sync.dma_start(out=mask_s[:, :], in_=m_e[blk * 128 : blk * 128 + 128, :])
                M_s = bp.tile([128, 1], F32, tag="M")
                nc.vector.tensor_reduce(M_s[:, :], scp[:, :], axis=AX.X, op=ALU.max)
                negM = bp.tile([128, 1], F32, tag="negM")
                nc.vector.tensor_scalar_mul(negM[:, :], M_s[:, :], -1.0)
                p_s = sp.tile([128, T], F32, tag="p")
                nc.scalar.activation(p_s[:, :], scp[:, :], AF.Exp, bias=negM[:, 0:1], scale=1.0)
                pm_s = sp.tile([128, T], F32, tag="pm")
                Z_s = bp.tile([128, 1], F32, tag="Z")
                nc.vector.scalar_tensor_tensor(
                    pm_s[:, :], p_s[:, :], 1.0, mask_s[:, :],
                    op0=ALU.mult, op1=ALU.mult, accum_out=Z_s[:, 0:1],
                )
                rZ = bp.tile([128, 1], F32, tag="rZ")
                nc.vector.reciprocal(rZ[:, :], Z_s[:, :])

                # transpose attn back to [t, b] columns
                pTh_p = pac.tile([128, 128], F32, tag="accA")
                nc.tensor.transpose(pTh_p[:, :], pm_s[:, 0:128], id_s[:, :])
                pTl_p = pac.tile([128, 128], F32, tag="accB")
                nc.tensor.transpose(pTl_p[0:72, :], pm_s[:, 128:200], id_s[:, :])
                pTh_s = bp.tile([128, 128], F32, tag="scTh")
                nc.vector.tensor_copy(pTh_s[:, :], pTh_p[:, :])
                pTl_s = bp.tile([128, 128], F32, tag="scTl")
                nc.vector.tensor_copy(pTl_s[0:72, :], pTl_p[0:72, :])

                # ---- attn @ v, output as columns [d, j] ----
                oT = pot.tile([D, 128], F32, tag="outT")
                for j in range(128):
                    b = blk * 128 + j
                    vh = vp.tile([128, D], F32, tag="vh")
                    nc.sync.dma_start(out=vh[:, :], in_=v_e[b, 0:128, :])
                    vl = vp.tile([128, D], F32, tag="vl")
                    nc.sync.dma_start(out=vl[0:72, :], in_=v_e[b, 128:200, :])
                    nc.tensor.matmul(
                        oT[:, j : j + 1], vh[:, :], pTh_s[:, j : j + 1], start=True, stop=False
                    )
                    nc.tensor.matmul(
                        oT[:, j : j + 1], vl[0:72, :], pTl_s[0:72, j : j + 1],
                        start=False, stop=True,
                    )
                oT_s = bp.tile([D, 128], F32, tag="oTs")
                nc.vector.tensor_copy(oT_s[:, :], oT[:, :])
                oN = pna.tile([128, D], F32, tag="nat")
                nc.tensor.transpose(oN[:, :], oT_s[:, :], id_s[0:D, 0:D])
                out_s = sp.tile([128, D], F32, tag="outs")
                nc.vector.tensor_scalar_mul(out_s[:, :], oN[:, :], rZ[:, 0:1])
                nc.sync.dma_start(
                    out=out_e[blk * 128 : blk * 128 + 128, :], in_=out_s[:, :]
                )
    nc.compile()
    return nc


def kernel(query, key, value, mask, W1, b1, W2, b2, Wo, bo, **kw):
    global _cached
    query = np.asarray(query, dtype=np.float32)
    key = np.asarray(key, dtype=np.float32)
    value = np.asarray(value, dtype=np.float32)
    mask_f = np.asarray(mask).astype(np.float32)
    W1 = np.asarray(W1, dtype=np.float32)

    W1a, W1b, W1c, W1d = W1[0:D], W1[D : 2 * D], W1[2 * D : 3 * D], W1[3 * D : 4 * D]
    Wk = np.ascontiguousarray(W1b - W1c)
    Wq = np.ascontiguousarray(W1a + W1c)
    Wd = np.ascontiguousarray(W1d)
    kT = np.ascontiguousarray(key.transpose(0, 2, 1))  # [B, D, T]
    ident = np.eye(128, dtype=np.float32)
    b1r = np.asarray(b1, dtype=np.float32).reshape(H1, 1)
    b2r = np.asarray(b2, dtype=np.float32).reshape(H2, 1)
    Wor = np.asarray(Wo, dtype=np.float32).reshape(H2, 1)
    # bo is a constant shift on all scores -> softmax-invariant; dropped.

    if _cached is None:
        _cached = build_nc()
    nc = _cached

    in_maps = []
    for c in range(NCORES):
        s = slice(c * BLOC, (c + 1) * BLOC)
        in_maps.append(
            {
                "qT": np.ascontiguousarray(query[s].T),
                "kT": kT[s],
                "v": value[s],
                "maskf": mask_f[s],
                "Wk": Wk,
                "Wd": Wd,
                "Wq": Wq,
                "b1": b1r,
                "W2": np.asarray(W2, dtype=np.float32),
                "b2": b2r,
                "Wo": Wor,
                "ident": ident,
            }
        )
    global _last_in_maps
    _last_in_maps = in_maps
    res = run_bass_kernel_spmd(nc, in_maps, list(range(NCORES)))
    outs = res.results if hasattr(res, "results") else res
    full = np.empty((B, D), dtype=np.float32)
    for c in range(NCORES):
        r = outs[c]
        arr = r["out"] if isinstance(r, dict) else r
        full[c * BLOC : (c + 1) * BLOC] = np.asarray(arr).reshape(BLOC, D)
    return full



# revision 24
# speedup vs baseline: 1.3453x; 1.3453x over previous
import sys

sys.path.insert(0, "/opt/trn_rl_repo")

import numpy as np
import ml_dtypes
import concourse.bass as bass
import concourse.bacc as bacc
import concourse.mybir as mybir
from concourse.tile import TileContext
from concourse.bass_utils import run_bass_kernel_spmd

B, T, D = 2048, 200, 64
H1, H2 = 128, 64
NCORES = 8
BLOC = B // NCORES  # 256 batches per core
NBLK = 2  # blocks of 128 batches
NPAIR = BLOC // 2  # 128 batch-pairs per core
BCH = 32  # batches per k chunk
NCH = BLOC // BCH  # 8 chunks per core
KLOOK = 3  # chunk prefetch depth (ring bufs = KLOOK + 1)

F32 = mybir.dt.float32
BF16 = mybir.dt.bfloat16
AF = mybir.ActivationFunctionType
ALU = mybir.AluOpType
AX = mybir.AxisListType

# relu1 per 16 pairs / relu2 per 16 quads: Act vs DVE
R1_PATTERN = ["A", "D", "A", "A", "D", "A", "A", "D", "A", "A", "D", "A", "A", "D", "A", "D"]
R2_PATTERN = ["A", "D", "A", "A", "D", "A", "A", "D", "A", "A", "D", "A", "A", "D", "A", "D"]
# qk half-chunk ops (16 per core): Pool vs DVE
QK_PATTERN = ["P", "D", "D", "D", "P", "D", "D", "D", "P", "D", "D", "D", "P", "D", "D", "D"]

_cached = None

import os

TRUNC = int(os.environ.get("BASS_TRUNC", "9"))


def build_nc():
    nc = bacc.Bacc()
    kD_e = nc.declare_dram_parameter("kD", [128, BLOC * T], BF16, isOutput=False)
    vhi_e = nc.declare_dram_parameter("vhi", [128, BLOC * D], BF16, isOutput=False)
    vlo_e = nc.declare_dram_parameter("vlo", [72, BLOC * D], BF16, isOutput=False)
    mP_e = nc.declare_dram_parameter("maskP", [64, NBLK * 2 * T], BF16, isOutput=False)
    qQ_e = nc.declare_dram_parameter("qQ", [128, BLOC], BF16, isOutput=False)
    qQf_e = nc.declare_dram_parameter("qQf", [64, BLOC], F32, isOutput=False)
    Wkd_e = nc.declare_dram_parameter("Wkd", [128, H1], BF16, isOutput=False)
    Wqb_e = nc.declare_dram_parameter("Wqb", [D, H1], BF16, isOutput=False)
    W2_e = nc.declare_dram_parameter("W2", [H1, H2], BF16, isOutput=False)
    Wo2_e = nc.declare_dram_parameter("Wo2", [128, 512], BF16, isOutput=False)
    b1_e = nc.declare_dram_parameter("b1", [H1, 1], F32, isOutput=False)
    b2P_e = nc.declare_dram_parameter("b2P", [128, 1], F32, isOutput=False)
    id_e = nc.declare_dram_parameter("ident", [64, 64], F32, isOutput=False)
    idb_e = nc.declare_dram_parameter("identb", [64, 64], BF16, isOutput=False)
    out_e = nc.declare_dram_parameter("out", [BLOC, D], F32, isOutput=True)
    Z_e = nc.declare_dram_parameter("Zout", [64, NBLK * 2], F32, isOutput=True)

    KCOLS = BCH * T  # 6400

    with TileContext(nc) as tc:
        with (
            tc.tile_pool(name="const", bufs=1) as cp,
            tc.tile_pool(name="kpool", bufs=KLOOK + 1) as kp,
            tc.tile_pool(name="vpool", bufs=2) as vp,
            tc.tile_pool(name="h1pool", bufs=4) as hp,
            tc.tile_pool(name="h2pool", bufs=3) as hp2,
            tc.tile_pool(name="bpool", bufs=2) as bp,
            tc.tile_pool(name="spool", bufs=2) as sp,
            tc.tile_pool(name="ps_h1", bufs=3, space="PSUM") as ph1,
            tc.tile_pool(name="ps_h2", bufs=2, space="PSUM") as ph2,
            tc.tile_pool(name="ps_acc", bufs=1, space="PSUM") as pac,
            tc.tile_pool(name="ps_nat", bufs=1, space="PSUM") as pna,
        ):
            kch_t = [None] * NCH

            def dma_kch(c):
                kc = kp.tile([128, KCOLS], BF16, tag="kD", name="kDc")
                kch_t[c] = kc
                nc.sync.dma_start(out=kc[:, :], in_=kD_e[:, c * KCOLS : (c + 1) * KCOLS])

            def emit_qk(c):
                # rows 64:128 *= q_b in place, per batch: per-partition scalar
                # runs at 4x on DVE for all-SBUF bf16
                kc = kch_t[c]
                for bb in range(BCH):
                    b = c * BCH + bb
                    tgt = kc[64:128, bb * T : (bb + 1) * T]
                    nc.vector.tensor_scalar_mul(tgt, tgt, qQf_s[:, b : b + 1])

            dma_kch(0)
            # consts needed early
            qQ_s = cp.tile([128, BLOC], BF16, tag="qQ")
            nc.sync.dma_start(out=qQ_s[:, :], in_=qQ_e[:, :])
            qQf_s = cp.tile([64, BLOC], F32, tag="qQf")
            nc.sync.dma_start(out=qQf_s[:, :], in_=qQf_e[:, :])
            Wkd_s = cp.tile([128, H1], BF16, tag="Wkd")
            nc.sync.dma_start(out=Wkd_s[:, :], in_=Wkd_e[:, :])
            Wqb_s = cp.tile([D, H1], BF16, tag="Wqb")
            nc.sync.dma_start(out=Wqb_s[:, :], in_=Wqb_e[:, :])
            b1_s = cp.tile([H1, 1], F32, tag="b1")
            nc.sync.dma_start(out=b1_s[:, :], in_=b1_e[:, :])
            for c in range(1, KLOOK + 1):
                dma_kch(c)
            W2_s = cp.tile([H1, H2], BF16, tag="W2")
            nc.sync.dma_start(out=W2_s[:, :], in_=W2_e[:, :])
            Wo2_s = cp.tile([128, 512], BF16, tag="Wo2")
            nc.sync.dma_start(out=Wo2_s[:, :], in_=Wo2_e[:, :])
            b2P_s = cp.tile([128, 1], F32, tag="b2P")
            nc.sync.dma_start(out=b2P_s[:, :], in_=b2P_e[:, :])
            id_s = cp.tile([64, 64], F32, tag="ident")
            nc.sync.dma_start(out=id_s[:, :], in_=id_e[:, :])
            idb_s = cp.tile([64, 64], BF16, tag="identb")
            nc.sync.dma_start(out=idb_s[:, :], in_=idb_e[:, :])

            def dma_vm(blk):
                vh = vp.tile([128, 128 * D], BF16, tag="vh", name="vh")
                nc.sync.dma_start(out=vh[:, :], in_=vhi_e[:, blk * 128 * D : (blk + 1) * 128 * D])
                vl = vp.tile([72, 128 * D], BF16, tag="vl", name="vl")
                nc.sync.dma_start(out=vl[:, :], in_=vlo_e[:, blk * 128 * D : (blk + 1) * 128 * D])
                mask_s = sp.tile([64, 2 * T], BF16, tag="mask", name="mask_s")
                nc.sync.dma_start(out=mask_s[:, :], in_=mP_e[:, blk * 2 * T : (blk + 1) * 2 * T])
                return vh, vl, mask_s

            vm_all = [dma_vm(b2) for b2 in range(NBLK)]
            for c in range(KLOOK + 1):
                emit_qk(c)

            next_ch = KLOOK + 1
            for blk in range(NBLK):
                vh, vl, mask_s = vm_all[blk]

                # scores for the whole block: row p = pair, cols (e, t)
                sc_all = pac.tile([64, 2 * T], F32, tag="accA", name="sc_all")

                h1p_t = [None] * 64
                h1s_t = [None] * 64
                h2q_t = [None] * 32
                h2s_t = [None] * 32

                def emit_l1(pr):
                    bg0 = blk * 128 + pr * 2
                    h1p = ph1.tile([128, 2 * T], F32, tag="h1", name="h1p")
                    h1p_t[pr] = h1p
                    for eh in range(2):
                        bg = bg0 + eh
                        kc = kch_t[bg // BCH]
                        co = (bg % BCH) * T
                        cs = slice(eh * T, (eh + 1) * T)
                        nc.tensor.matmul(
                            h1p[:, cs], Wkd_s[:, :], kc[:, co : co + T],
                            start=True, stop=False,
                        )
                        nc.tensor.matmul(
                            h1p[:, cs], Wqb_s[:, :],
                            qQ_s[0:64, bg : bg + 1].broadcast_to([64, T]),
                            start=False, stop=True,
                        )

                def emit_relu1(pr):
                    h1s = hp.tile([128, 2 * T], BF16, tag="h1s", name="h1s")
                    h1s_t[pr] = h1s
                    h1p = h1p_t[pr]
                    eng = R1_PATTERN[(blk * 64 + pr) % 16]
                    if eng == "A":
                        nc.scalar.activation(
                            h1s[:, :], h1p[:, :], AF.Relu, bias=b1_s[:, 0:1], scale=1.0
                        )
                    else:
                        nc.vector.tensor_scalar(
                            h1s[:, :], h1p[:, :], b1_s[:, 0:1], 0.0, op0=ALU.add, op1=ALU.max
                        )
                    h1p_t[pr] = None

                def emit_l2(pr):
                    qd, half = pr // 2, pr % 2
                    if half == 0:
                        h2q_t[qd] = ph2.tile([128, 2 * T], F32, tag="h2", name="h2q")
                    nc.tensor.matmul(
                        h2q_t[qd][half * 64 : (half + 1) * 64, :], W2_s[:, :],
                        h1s_t[pr][:, :], start=True, stop=True,
                        skip_group_check=True,
                    )

                def emit_relu2(qd):
                    h2s = hp2.tile([128, 2 * T], BF16, tag="h2s", name="h2s")
                    h2s_t[qd] = h2s
                    if R2_PATTERN[qd % 16] == "A":
                        nc.scalar.activation(
                            h2s[:, :], h2q_t[qd][:, :], AF.Relu, bias=b2P_s[:, 0:1], scale=1.0
                        )
                    else:
                        nc.vector.tensor_scalar(
                            h2s[:, :], h2q_t[qd][:, :], b2P_s[:, 0:1], 0.0,
                            op0=ALU.add, op1=ALU.max,
                        )
                    h2q_t[qd] = None

                def emit_score(qd):
                    # 16 quads accumulate into one 32-partition group; the
                    # shifted stationary is zero outside this quad's 2 rows
                    g, m = qd // 16, qd % 16
                    nc.tensor.matmul(
                        sc_all[g * 32 : g * 32 + 32, :],
                        Wo2_s[:, m * 32 : (m + 1) * 32],
                        h2s_t[qd][:, :],
                        start=(m == 0), stop=(m == 15),
                        skip_group_check=True,
                    )

                for i in range(64 + 8):
                    if i < 64 and i % 16 == 0 and next_ch < NCH:
                        dma_kch(next_ch)
                        emit_qk(next_ch)
                        next_ch += 1
                    if TRUNC >= 2 and i < 64:
                        emit_l1(i)
                    if TRUNC >= 2 and 1 <= i < 65:
                        emit_relu1(i - 1)
                    if TRUNC >= 3 and 2 <= i < 66:
                        emit_l2(i - 2)
                    if TRUNC >= 3 and 3 <= i < 67 and (i - 3) % 2 == 1:
                        emit_relu2((i - 3) // 2)
                    if TRUNC >= 4 and 5 <= i < 69 and (i - 5) % 2 == 0:
                        emit_score((i - 5) // 2)

                if TRUNC < 5:
                    out_s0 = sp.tile([128, D], F32, tag="outs", name="out_s0")
                    nc.vector.memset(out_s0[:, :], 0.0)
                    nc.sync.dma_start(
                        out=out_e[blk * 128 : blk * 128 + 128, :], in_=out_s0[:, :]
                    )
                    Zs0 = bp.tile([64, 2], F32, tag="Zs", name="Zs0")
                    nc.vector.memset(Zs0[:, :], 1.0)
                    nc.sync.dma_start(out=Z_e[:, blk * 2 : (blk + 1) * 2], in_=Zs0[:, :])
                    continue

                # ---- softmax (no max subtraction; scores are O(3)) ----
                pmx = sp.tile([64, 2 * T], BF16, tag="pmx", name="pmx")
                nc.scalar.activation(pmx[:, :], sc_all[:, :], AF.Exp, bias=0.0, scale=1.0)
                pmm = sp.tile([64, 2 * T], BF16, tag="pmm", name="pmm")
                nc.vector.tensor_tensor(pmm[:, :], pmx[:, :], mask_s[:, :], op=ALU.mult)
                Zs = bp.tile([64, 2], F32, tag="Zs", name="Zs")
                nc.vector.tensor_reduce(Zs[:, 0:1], pmm[:, 0:T], axis=AX.X, op=ALU.add)
                nc.vector.tensor_reduce(Zs[:, 1:2], pmm[:, T : 2 * T], axis=AX.X, op=ALU.add)
                nc.sync.dma_start(out=Z_e[:, blk * 2 : (blk + 1) * 2], in_=Zs[:, :])

                if TRUNC < 6:
                    out_s0 = sp.tile([128, D], F32, tag="outs", name="out_s0")
                    nc.vector.memset(out_s0[:, :], 0.0)
                    nc.sync.dma_start(
                        out=out_e[blk * 128 : blk * 128 + 128, :], in_=out_s0[:, :]
                    )
                    continue

                # transpose attn to [t, pair]; cols [hi_e0 | hi_e1 | lo_e0 | lo_e1]
                pT_p = pna.tile([128, 256], BF16, tag="natb", name="pT_p")
                nc.tensor.transpose(pT_p[0:128, 0:64], pmm[:, 0:128], idb_s[:, :])
                nc.tensor.transpose(pT_p[0:128, 64:128], pmm[:, 200:328], idb_s[:, :])
                nc.tensor.transpose(pT_p[0:72, 128:192], pmm[:, 128:200], idb_s[:, :])
                nc.tensor.transpose(pT_p[0:72, 192:256], pmm[:, 328:400], idb_s[:, :])
                pT_s = bp.tile([128, 256], BF16, tag="pTs", name="pT_s")
                nc.vector.tensor_copy(pT_s[0:128, 0:128], pT_p[0:128, 0:128])
                nc.vector.tensor_copy(pT_s[0:72, 128:256], pT_p[0:72, 128:256])

                # ---- attn @ v, output as columns [d, j] ----
                oT = pna.tile([D, 128], F32, tag="nat", name="oT")
                for j in range(128):
                    p, e = j // 2, j % 2
                    nc.tensor.matmul(
                        oT[:, j : j + 1], vh[:, j * D : (j + 1) * D],
                        pT_s[0:128, 64 * e + p : 64 * e + p + 1],
                        start=True, stop=False,
                    )
                    nc.tensor.matmul(
                        oT[:, j : j + 1], vl[:, j * D : (j + 1) * D],
                        pT_s[0:72, 128 + 64 * e + p : 128 + 64 * e + p + 1],
                        start=False, stop=True,
                    )
                oT_s = bp.tile([D, 128], F32, tag="oTs")
                nc.vector.tensor_copy(oT_s[:, :], oT[:, :])
                oN = pna.tile([128, D], F32, tag="nat", name="oN")
                nc.tensor.transpose(oN[:, :], oT_s[:, :], id_s[:, :])
                out_s = sp.tile([128, D], F32, tag="outs")
                nc.vector.tensor_copy(out_s[:, :], oN[:, :])
                nc.sync.dma_start(
                    out=out_e[blk * 128 : blk * 128 + 128, :], in_=out_s[:, :]
                )
    nc.compile()
    return nc


def kernel(query, key, value, mask, W1, b1, W2, b2, Wo, bo, **kw):
    global _cached
    query = np.asarray(query, dtype=np.float32)
    key = np.asarray(key, dtype=np.float32)
    value = np.asarray(value, dtype=np.float32)
    mask_f = np.asarray(mask).astype(np.float32)
    W1 = np.asarray(W1, dtype=np.float32)

    W1a, W1b, W1c, W1d = W1[0:D], W1[D : 2 * D], W1[2 * D : 3 * D], W1[3 * D : 4 * D]
    Wk = W1b - W1c
    Wq = W1a + W1c
    Wd = W1d
    Wkd = np.ascontiguousarray(np.vstack([Wk, Wd])).astype(ml_dtypes.bfloat16)
    Wqb = np.ascontiguousarray(Wq).astype(ml_dtypes.bfloat16)
    W2b = np.asarray(W2, dtype=np.float32).astype(ml_dtypes.bfloat16)
    Wor = np.asarray(Wo, dtype=np.float32).reshape(H2)
    Wo2 = np.zeros((128, 16, 32), dtype=np.float32)
    for m in range(16):
        Wo2[0:64, m, 2 * m] = Wor
        Wo2[64:128, m, 2 * m + 1] = Wor
    Wo2 = Wo2.reshape(128, 512).astype(ml_dtypes.bfloat16)
    b1r = np.asarray(b1, dtype=np.float32).reshape(H1, 1)
    b2r = np.asarray(b2, dtype=np.float32).reshape(H2, 1)
    b2P = np.ascontiguousarray(np.vstack([b2r, b2r]))
    ident = np.eye(64, dtype=np.float32)
    # bo is a constant shift on all scores -> softmax-invariant; dropped.

    if _cached is None:
        _cached = build_nc()
    nc = _cached

    in_maps = []
    for c in range(NCORES):
        s = slice(c * BLOC, (c + 1) * BLOC)
        keyc = key[s]  # [256, 200, 64]
        valc = value[s]
        kT = keyc.transpose(2, 0, 1).reshape(D, BLOC * T).astype(ml_dtypes.bfloat16)
        kD = np.vstack([kT, kT])
        vhi = (
            valc[:, 0:128, :].transpose(1, 0, 2).reshape(128, BLOC * D).astype(ml_dtypes.bfloat16)
        )
        vlo = (
            valc[:, 128:200, :].transpose(1, 0, 2).reshape(72, BLOC * D).astype(ml_dtypes.bfloat16)
        )
        qT = query[s].T.astype(ml_dtypes.bfloat16)  # [64, 256]
        qQ = np.vstack([qT, qT])
        # maskP[p, blk*400 + e*200 + t] = mask[blk*128 + 2p + e, t]
        maskP = (
            mask_f[s]
            .reshape(NBLK, 64, 2, T)
            .transpose(1, 0, 2, 3)
            .reshape(64, NBLK * 2 * T)
            .astype(ml_dtypes.bfloat16)
        )
        in_maps.append(
            {
                "kD": np.ascontiguousarray(kD),
                "vhi": np.ascontiguousarray(vhi),
                "vlo": np.ascontiguousarray(vlo),
                "maskP": np.ascontiguousarray(maskP),
                "qQ": np.ascontiguousarray(qQ),
                "qQf": np.ascontiguousarray(query[s].T.astype(np.float32)),
                "Wkd": Wkd,
                "Wqb": Wqb,
                "W2": W2b,
                "Wo2": Wo2,
                "b1": b1r,
                "b2P": b2P,
                "ident": ident,
                "identb": ident.astype(ml_dtypes.bfloat16),
            }
        )
    global _last_in_maps
    _last_in_maps = in_maps
    res = run_bass_kernel_spmd(nc, in_maps, list(range(NCORES)))
    outs = res.results if hasattr(res, "results") else res
    full = np.empty((B, D), dtype=np.float32)
    for c in range(NCORES):
        r = outs[c]
        o = np.asarray(r["out"]).reshape(BLOC, D)
        Zo = np.asarray(r["Zout"]).reshape(64, NBLK, 2)
        Z = Zo.transpose(1, 0, 2).reshape(BLOC)  # [blk, p, e] -> b
        full[c * BLOC : (c + 1) * BLOC] = o / Z[:, None]
    return full
